# revision 18
# baseline (speedup 1.0000x reference)
"""Self-contained Trainium2 Bass kernel for the BasicAttentionBlock problem.

Full inputs in, full outputs out. 8 NeuronCores, data-parallel over
(batch element x query-half): each core computes GroupNorm + q/k/v 1x1
convs + full-key attention for its 2048 query pixels + output projection
+ residual, entirely on-chip.

Design notes (v2):
- GroupNorm is folded into the conv weights on-chip (w' = w * a per input
  channel, conv biases recomputed from the GN shift b), so the convs
  consume raw x and the stats -> first-matmul chain is short. The k-conv
  bias cancels in softmax and is dropped.
- exp(S^T) on ACT (the bottleneck: 65536 columns/core) writes fp8 pT.
- AV and the softmax denominator are fp8 DoubleRow matmuls over key-block
  pairs (0.5 cycles/row): the denominator costs 16 matmuls/block on PE
  instead of a 31-add Pool tree, and AV halves.
- 1/den is broadcast across partitions by gpsimd; y is scaled by 1/den
  before the projection conv so the epilogue chain is short.
- PSUM: tag 'st' = 2 x [128,3,512] S^T groups (12KB), tag 'u' = 2 x 2KB
  rotating everything else (conv chunks, AV accumulators, denominators,
  projections) in a hand-ordered schedule that keeps the rotation free
  of slot deadlocks.
"""

import numpy as np

B = 4
C = 128
H = 64
W = 64
HW = H * W          # 4096
HALF = HW // 2      # 2048 query pixels per core
NCORES = 8
GROUPS = 8
GSIZE = C // GROUPS  # 16
EPS = 1e-5
SCL = 1.0 / np.sqrt(C)   # attention logit scale
NPIX_G = GSIZE * HW      # elements per group-norm group = 65536

_CACHE = {}


def _split_excess_waits(nc, limit=1):
    """Rewrite instructions so none carries more than `limit` sync-waits.

    The walrus build in this container rejects instructions with more than
    one sync-wait command ("Too many sync wait commands"), while Tile's
    semaphore assignment freely attaches several. Excess waits are hoisted
    onto standalone InstEventSemaphore instructions placed immediately
    before the owning instruction on the same engine queue — semantically
    identical (program order on one engine), just more instructions.
    """
    import concourse.mybir as mybir

    ctr = 0
    for f in nc.m.functions:
        for bb in f.blocks:
            new = []
            changed = False
            for inst in bb.instructions:
                si = getattr(inst, "sync_info", None)
                ow = list(si.on_wait) if si is not None else []
                if len(ow) > limit:
                    # keep register-valued waits on the original instruction
                    imm = [w for w in ow if w.wait_reg is None]
                    reg = [w for w in ow if w.wait_reg is not None]
                    keep_n = max(0, limit - len(reg))
                    hoist = imm[: len(imm) - keep_n] if keep_n < len(imm) else []
                    kept = reg + imm[len(imm) - keep_n :] if keep_n else reg
                    assert len(kept) <= max(limit, len(reg))
                    for w in hoist:
                        ev = mybir.InstEventSemaphore(
                            name=f"waitsplit_{ctr}", ins=[], outs=[]
                        )
                        ctr += 1
                        ev.engine = inst.engine
                        ev.sync_info = mybir.SyncInfo(on_wait=[w], on_update=[])
                        nc.register_instruction(ev, overwrite=True)
                        new.append(ev)
                    si.on_wait = kept
                    inst.sync_info = si
                    changed = True
                new.append(inst)
            if changed:
                bb.instructions = new


def _build_bass():
    import concourse.bass as bass
    import concourse.mybir as mybir

    fp32 = mybir.dt.float32
    bf16 = mybir.dt.bfloat16
    fp8 = mybir.dt.float8e4
    AF = mybir.ActivationFunctionType
    ALU = mybir.AluOpType
    AX = mybir.AxisListType
    DR = mybir.MatmulPerfMode.DoubleRow
    from concourse.tile import TileContext as TC

    nc = bass.Bass(trn_type="TRN2")

    # ---- I/O -----------------------------------------------------------
    x_d = nc.dram_tensor("x", [C, HALF], fp32, kind="ExternalInput")
    xbf_d = nc.dram_tensor("x_bf", [C, HW], bf16, kind="ExternalInput")
    wq_d = nc.dram_tensor("wq_t", [C, C], bf16, kind="ExternalInput")
    wk_d = nc.dram_tensor("wk_t", [C, C], bf16, kind="ExternalInput")
    wv_d = nc.dram_tensor("wv_t", [C, C], bf16, kind="ExternalInput")
    wp_d = nc.dram_tensor("wp_t", [C, C], bf16, kind="ExternalInput")
    bq_d = nc.dram_tensor("bq", [C, 1], fp32, kind="ExternalInput")
    bv_d = nc.dram_tensor("bv", [C, 1], fp32, kind="ExternalInput")
    bp_d = nc.dram_tensor("bp", [C, 1], fp32, kind="ExternalInput")
    gnb_d = nc.dram_tensor("gn_b", [C, 1], fp32, kind="ExternalInput")
    gmat_d = nc.dram_tensor("gmat", [C, GROUPS], fp32, kind="ExternalInput")
    gbc_d = nc.dram_tensor("gbc", [GROUPS, C], fp32, kind="ExternalInput")
    ones2_d = nc.dram_tensor("ones2", [C, 2], fp8, kind="ExternalInput")
    out_d = nc.dram_tensor("out", [C, HALF], fp32, kind="ExternalOutput")

    with TC(nc) as tc, tc.tile_pool(name="main", bufs=1) as pool, tc.tile_pool(
        name="psum", bufs=1, space="PSUM"
    ) as psum:
        # gpsimd 'proxy' library: tensor_tensor + partition_broadcast
        from concourse import library_config

        nc.gpsimd.load_library(library_config.proxy)

        # ---- ACT table prewarm (hide the exp table load) ---------------
        dum = pool.tile([1, 2], fp32, name="dum")
        nc.vector.memset(dum[:], 0.0)

        # ---- SBUF tiles -------------------------------------------------
        x_bf = pool.tile([C, HW], bf16, name="x_bf")
        x_sb = pool.tile([C, HALF], fp32, name="x_sb")
        wq_sb = pool.tile([C, C], bf16, name="wq_sb")
        wk_sb = pool.tile([C, C], bf16, name="wk_sb")
        wv_sb = pool.tile([C, C], bf16, name="wv_sb")
        wp_sb = pool.tile([C, C], bf16, name="wp_sb")
        wqs_sb = pool.tile([C, C], bf16, name="wqs_sb")
        wks_sb = pool.tile([C, C], bf16, name="wks_sb")
        wvs_sb = pool.tile([C, C], bf16, name="wvs_sb")
        bq_sb = pool.tile([C, 1], fp32, name="bq_sb")
        bv_sb = pool.tile([C, 1], fp32, name="bv_sb")
        bp_sb = pool.tile([C, 1], fp32, name="bp_sb")
        gnb_sb = pool.tile([C, 1], fp32, name="gnb_sb")
        gmat_sb = pool.tile([C, GROUPS], fp32, name="gmat_sb")
        gbc_sb = pool.tile([GROUPS, C], fp32, name="gbc_sb")
        ones2_sb = pool.tile([C, 2], fp8, name="ones2_sb")

        # ---- DMAs -------------------------------------------------------
        # SP queue: x_bf chunks 0-3 (stats-critical), then weights/consts
        # in order of first use; ACT queue: x_bf 4-5; Pool queue: x_bf
        # 6-7, then the late consts + residual x.
        for c4 in range(2):
            sl = slice(1024 * c4, 1024 * (c4 + 1))
            nc.sync.dma_start(x_bf[:, sl], xbf_d[:, sl])
        nc.scalar.dma_start(x_bf[:, 2048:3072], xbf_d[:, 2048:3072])
        nc.gpsimd.dma_start(x_bf[:, 3072:4096], xbf_d[:, 3072:4096])
        nc.sync.dma_start(wq_sb[:], wq_d[:])
        nc.sync.dma_start(wk_sb[:], wk_d[:])
        nc.sync.dma_start(gmat_sb[:], gmat_d[:])
        nc.sync.dma_start(gbc_sb[:], gbc_d[:])
        nc.sync.dma_start(gnb_sb[:], gnb_d[:])
        nc.sync.dma_start(bq_sb[:], bq_d[:])
        nc.sync.dma_start(wv_sb[:], wv_d[:])
        nc.sync.dma_start(wp_sb[:], wp_d[:])
        # prewarm exp/ln table while the stats DMAs stream
        nc.scalar.activation(dum[:], dum[:], AF.Exp)

        # ---- GroupNorm stats (chunked, overlaps the x DMA) -------------
        # per-channel sums on Pool (idle, no access-latency penalty);
        # per-channel sum of squares: first half on ACT (Square + accum,
        # shares the exp table set), second half on DVE (ttr).
        s_parts = pool.tile([C, 8], fp32, name="s_parts")
        ss_parts = pool.tile([C, 2], fp32, name="ss_parts")
        sq_scr = pool.tile([C, 2048], bf16, name="sq_scr")
        tt_scr = pool.tile([C, 2048], bf16, name="tt_scr")
        s_scr = pool.tile([C, 512], bf16, name="s_scr")
        stats = pool.tile([C, 2], fp32, name="stats")
        for ci in (6, 7, 4, 5, 0, 1, 2, 3):  # in order of chunk arrival
            sl = slice(512 * ci, 512 * (ci + 1))
            nc.gpsimd.tensor_scalar(
                s_scr[:],
                x_bf[:, sl],
                0.0,
                None,
                ALU.add,
                ALU.add,
                accum_out=s_parts[:, ci : ci + 1],
            )
        nc.scalar.activation(
            sq_scr[:], x_bf[:, 0:2048], AF.Square, accum_out=ss_parts[:, 0:1]
        )
        nc.vector.tensor_tensor_reduce(
            tt_scr[:],
            x_bf[:, 2048:4096],
            x_bf[:, 2048:4096],
            1.0,
            0.0,
            ALU.mult,
            ALU.add,
            accum_out=ss_parts[:, 1:2],
        )
        nc.vector.tensor_reduce(stats[:, 0:1], s_parts[:], axis=AX.X, op=ALU.add)
        nc.vector.tensor_reduce(stats[:, 1:2], ss_parts[:], axis=AX.X, op=ALU.add)

        # late consts + residual x on the Pool queue, AFTER the stats sums
        nc.gpsimd.dma_start(bv_sb[:], bv_d[:])
        nc.gpsimd.dma_start(bp_sb[:], bp_d[:])
        nc.gpsimd.dma_start(ones2_sb[:], ones2_d[:])
        for c4 in range(4):
            sl = slice(512 * c4, 512 * (c4 + 1))
            nc.gpsimd.dma_start(x_sb[:, sl], x_d[:, sl])

        eps_sb = pool.tile([GROUPS, 1], fp32, name="eps_sb")
        nc.vector.memset(eps_sb[:], EPS)

        gsum_ps = psum.tile([GROUPS, 2], fp32, name="gsum_ps", tag="u", bufs=2)
        nc.tensor.matmul(gsum_ps[:], gmat_sb[:], stats[:], start=True, stop=True)
        me2 = pool.tile([GROUPS, 2], fp32, name="me2")
        nc.vector.tensor_copy(me2[:], gsum_ps[:])

        msq = pool.tile([GROUPS, 1], fp32, name="msq")
        nc.vector.tensor_tensor(msq[:], me2[:, 0:1], me2[:, 0:1], ALU.mult)
        tve = pool.tile([GROUPS, 1], fp32, name="tve")
        nc.vector.tensor_tensor(tve[:], me2[:, 1:2], msq[:], ALU.subtract)

        # rsqrt(var+eps) = exp(-0.5*ln(var+eps)); eps rides the Ln bias.
        lnt = pool.tile([GROUPS, 1], fp32, name="lnt")
        nc.scalar.activation(lnt[:], tve[:], AF.Ln, bias=eps_sb[:])
        r1 = pool.tile([GROUPS, 1], fp32, name="r1")
        nc.scalar.activation(r1[:], lnt[:], AF.Exp, scale=-0.5)
        mr = pool.tile([GROUPS, 1], fp32, name="mr")
        nc.vector.tensor_tensor(mr[:], me2[:, 0:1], r1[:], ALU.mult)

        # a = gn_w * rsqrt (per channel), b = gn_b - mean * a
        a_ps = psum.tile([C, 1], fp32, name="a_ps", tag="u", bufs=2)
        nc.tensor.matmul(a_ps[:], gbc_sb[:], r1[:], start=True, stop=True)
        bm_ps = psum.tile([C, 1], fp32, name="bm_ps", tag="u", bufs=2)
        nc.tensor.matmul(bm_ps[:], gbc_sb[:], mr[:], start=True, stop=True)
        a_sb = pool.tile([C, 1], fp32, name="a_sb")
        nc.vector.tensor_copy(a_sb[:], a_ps[:])
        b_sb = pool.tile([C, 1], fp32, name="b_sb")
        nc.vector.tensor_tensor(b_sb[:], gnb_sb[:], bm_ps[:], ALU.subtract)

        # fold the GN scale into the conv weights: w'[c,o] = w_t[c,o]*a[c]
        nc.vector.tensor_scalar(wqs_sb[:], wq_sb[:], a_sb[:], None, ALU.mult)
        nc.vector.tensor_scalar(wks_sb[:], wk_sb[:], a_sb[:], None, ALU.mult)
        nc.vector.tensor_scalar(wvs_sb[:], wv_sb[:], a_sb[:], None, ALU.mult)

        # conv biases from the GN shift b:
        #   bhq = wq.b + bq ; bhv = wv.b + bv ; bp2 = wp.bhv + bp
        # (the k-conv bias is constant per query in the logits -> cancels)
        b_bf = pool.tile([C, 1], bf16, name="b_bf")
        nc.vector.tensor_copy(b_bf[:], b_sb[:])
        bhq_ps = psum.tile([C, 1], fp32, name="bhq_ps", tag="u", bufs=2)
        nc.tensor.matmul(bhq_ps[:], wq_sb[:], b_bf[:], start=True, stop=True)
        bhq_sb = pool.tile([C, 1], fp32, name="bhq_sb")
        nc.vector.tensor_tensor(bhq_sb[:], bhq_ps[:], bq_sb[:], ALU.add)

        bhv_ps = psum.tile([C, 1], fp32, name="bhv_ps", tag="u", bufs=2)
        nc.tensor.matmul(bhv_ps[:], wv_sb[:], b_bf[:], start=True, stop=True)
        bhv_sb = pool.tile([C, 1], fp32, name="bhv_sb")
        nc.vector.tensor_tensor(bhv_sb[:], bhv_ps[:], bv_sb[:], ALU.add)
        bhv_bf = pool.tile([C, 1], bf16, name="bhv_bf")
        nc.vector.tensor_copy(bhv_bf[:], bhv_sb[:])
        pb_ps = psum.tile([C, 1], fp32, name="pb_ps", tag="u", bufs=2)
        nc.tensor.matmul(pb_ps[:], wp_sb[:], bhv_bf[:], start=True, stop=True)
        bp2_sb = pool.tile([C, 1], fp32, name="bp2_sb")
        nc.vector.tensor_tensor(bp2_sb[:], pb_ps[:], bp_sb[:], ALU.add)
        # xb = x + bp2 (residual + folded projection bias), on Pool
        xb = pool.tile([C, HALF], fp32, name="xb")
        nc.gpsimd.tensor_scalar(xb[:], x_sb[:], bp2_sb[:], None, ALU.add)

        # ---- conv emitters ---------------------------------------------
        k_bf = pool.tile([C, HW], bf16, name="k_bf")
        q_bf = pool.tile([C, HALF], bf16, name="q_bf")
        vT_f8 = pool.tile([C, 32, C], fp8, name="vT_f8")

        def emit_k_chunk(c8):
            sl = slice(512 * c8, 512 * (c8 + 1))
            kps = psum.tile([C, 512], fp32, name=f"kps{c8}", tag="u", bufs=2)
            nc.tensor.matmul(kps[:], wks_sb[:], x_bf[:, sl], start=True, stop=True)
            nc.vector.tensor_copy(k_bf[:, sl], kps[:])

        def emit_q_chunk(c4):
            sl = slice(512 * c4, 512 * (c4 + 1))
            qps = psum.tile([C, 512], fp32, name=f"qps{c4}", tag="u", bufs=2)
            nc.tensor.matmul(qps[:], wqs_sb[:], x_bf[:, sl], start=True, stop=True)
            if c4 == 0:
                # ACT is idle pre-body: overlap the q evac with the k evac
                nc.scalar.activation(q_bf[:, sl], qps[:], AF.Identity, bias=bhq_sb[:])
            else:
                nc.vector.tensor_scalar(q_bf[:, sl], qps[:], bhq_sb[:], None, ALU.add)

        def emit_v_chunk(g8):
            vps = psum.tile([C, 512], fp32, name=f"vps{g8}", tag="u", bufs=2)
            for m in range(4):
                jb = 4 * g8 + m
                nc.tensor.matmul(
                    vps[:, 128 * m : 128 * (m + 1)],
                    x_bf[:, 128 * jb : 128 * (jb + 1)],
                    wvs_sb[:],
                    start=True,
                    stop=True,
                )
            nc.vector.tensor_copy(
                vT_f8[:, 4 * g8 : 4 * (g8 + 1), :],
                vps[:].rearrange("p (m c) -> p m c", m=4),
            )

        emit_q_chunk(0)
        emit_k_chunk(0)

        # ---- attention --------------------------------------------------
        jgroups = [(3 * g, 3) for g in range(10)] + [(30, 2)]
        n_ib = HALF // 512  # 4 query blocks of 512
        pT_tiles = [None] * n_ib
        yps_tiles = [None] * n_ib
        den_tiles = [None] * n_ib
        out_sb = pool.tile([C, HALF], fp32, name="out_sb")
        ones_ap = ones2_sb[:].unsqueeze(2)  # [128, 2, 1]

        conv_state = {"k": 1, "q": 1, "v": 0}
        # conv chunk emission schedule for block 0 (group -> jobs).
        blk0_jobs = {
            0: ["k", "v"], 1: ["k", "v"], 2: ["k", "v"], 3: ["k", "q", "v"],
            4: ["k", "v"], 5: ["k", "q", "v"], 6: ["k", "v"], 7: ["k", "q", "v"],
        }

        def run_conv_job(j):
            if j == "k" and conv_state["k"] < 8:
                emit_k_chunk(conv_state["k"])
                conv_state["k"] += 1
            elif j == "q" and conv_state["q"] < 4:
                emit_q_chunk(conv_state["q"])
                conv_state["q"] += 1
            elif j == "v" and conv_state["v"] < 8:
                emit_v_chunk(conv_state["v"])
                conv_state["v"] += 1

        def alloc_y(ib):
            yps_tiles[ib] = psum.tile([C, 512], fp32, name=f"yps{ib}", tag="u", bufs=2)

        def alloc_d(ib):
            den_tiles[ib] = psum.tile([1, 512], fp32, name=f"den{ib}", tag="u", bufs=2)

        def emit_av_pair(ib, p):
            nc.tensor.matmul(
                yps_tiles[ib][:],
                vT_f8[:, 2 * p : 2 * p + 2, :],
                pT_tiles[ib][:, 2 * p : 2 * p + 2, :],
                start=(p == 0),
                stop=(p == 15),
                perf_mode=DR,
            )

        def emit_den_pair(ib, p):
            nc.tensor.matmul(
                den_tiles[ib][:],
                ones_ap,
                pT_tiles[ib][:, 2 * p : 2 * p + 2, :],
                start=(p == 0),
                stop=(p == 15),
                perf_mode=DR,
            )

        rbc_tiles = [None] * n_ib
        ybf_tiles = [None] * n_ib

        def emit_epi_a(ib, hs=slice(0, 512), tag=""):
            """den -> 1/den -> partition-broadcast -> y*(1/den): emitted at
            the end of the block that ran AV(ib)/den(ib)."""
            n = hs.stop - hs.start
            rden = pool.tile([1, n], fp32, name=f"rden{ib}{tag}", tag="rden", bufs=2)
            nc.vector.reciprocal(rden[:], den_tiles[ib][:, hs])
            rbc = pool.tile([C, n], fp32, name=f"rbc{ib}{tag}", tag="rbc", bufs=2)
            nc.gpsimd.partition_broadcast(rbc[:], rden[:])
            y_bf = pool.tile([C, n], bf16, name=f"ybf{ib}{tag}", tag="ybf", bufs=2)
            nc.vector.tensor_tensor(y_bf[:], yps_tiles[ib][:, hs], rbc[:], ALU.mult)
            ybf_tiles[ib] = y_bf

        def emit_epi_b(ib, hs=slice(0, 512), tag=""):
            """proj -> + (x + bp2) -> store."""
            n = hs.stop - hs.start
            sl = slice(512 * ib + hs.start, 512 * ib + hs.stop)
            pps = psum.tile([C, n], fp32, name=f"pps{ib}{tag}", tag="u", bufs=2)
            nc.tensor.matmul(pps[:], wp_sb[:], ybf_tiles[ib][:], start=True, stop=True)
            nc.vector.tensor_tensor(out_sb[:, sl], pps[:], xb[:, sl], ALU.add)
            nc.sync.dma_start(out_d[:, sl], out_sb[:, sl])

        def emit_st_group(ib, j0, glen):
            st = psum.tile([C, glen, 512], fp32, name=f"st{ib}_{j0}", tag="st", bufs=2)
            qs = q_bf[:, 512 * ib : 512 * (ib + 1)]
            for u2 in range(glen):
                jb = j0 + u2
                nc.tensor.matmul(
                    st[:, u2, :],
                    k_bf[:, 128 * jb : 128 * (jb + 1)],
                    qs,
                    start=True,
                    stop=True,
                )
            nc.scalar.activation(
                pT_tiles[ib][:, j0 : j0 + glen, :], st[:], AF.Exp, scale=float(SCL)
            )

        # -- block 0: S^T/exp + remaining conv chunks ---------------------
        pT_tiles[0] = pool.tile([C, 32, 512], fp8, name="pT0", tag="pT", bufs=3)
        for gi, (j0, glen) in enumerate(jgroups):
            emit_st_group(0, j0, glen)
            for j in blk0_jobs.get(gi, []):
                run_conv_job(j)

        # -- block 1: + AV0/den0 spread over the block --------------------
        pT_tiles[1] = pool.tile([C, 32, 512], fp8, name="pT1", tag="pT", bufs=3)
        alloc_y(0)
        alloc_d(0)
        av_done = den_done = 0
        for gi, (j0, glen) in enumerate(jgroups):
            emit_st_group(1, j0, glen)
            if gi >= 1:
                tgt = min(16, 2 * gi)
                while av_done < tgt:
                    emit_av_pair(0, av_done)
                    av_done += 1
                while den_done < tgt:
                    emit_den_pair(0, den_done)
                    den_done += 1
        while av_done < 16:
            emit_av_pair(0, av_done)
            av_done += 1
        while den_done < 16:
            emit_den_pair(0, den_done)
            den_done += 1
        emit_epi_a(0)

        # -- block 2: + epilogue-B(0), AV1/den1 spread --------------------
        pT_tiles[2] = pool.tile([C, 32, 512], fp8, name="pT2", tag="pT", bufs=3)
        av_done = den_done = 0
        for gi, (j0, glen) in enumerate(jgroups):
            emit_st_group(2, j0, glen)
            if gi == 0:
                emit_epi_b(0)   # pps0 (u): after ymul0 read of yps0
                alloc_y(1)      # yps1 (u): after recip0 read of den0
            if gi >= 1:
                tgt = min(16, 2 * gi)
                while av_done < tgt:
                    emit_av_pair(1, av_done)
                    av_done += 1
            if gi == 2:
                alloc_d(1)      # den1 (u): after the out0 add read of pps0
            if gi >= 3:
                tgt = min(16, 3 * (gi - 2))
                while den_done < tgt:
                    emit_den_pair(1, den_done)
                    den_done += 1
        while av_done < 16:
            emit_av_pair(1, av_done)
            av_done += 1
        while den_done < 16:
            emit_den_pair(1, den_done)
            den_done += 1
        emit_epi_a(1)

        # -- block 3: epi-B(1), AV2/den2 bursts, self-trailing AV3/den3 --
        pT_tiles[3] = pool.tile([C, 32, 512], fp8, name="pT3", tag="pT", bufs=3)
        av2 = den2 = 0
        av3 = den3 = 0
        for gi, (j0, glen) in enumerate(jgroups):
            emit_st_group(3, j0, glen)
            if gi == 0:
                emit_epi_b(1)   # pps1 (u): after ymul1 read of yps1
                alloc_y(2)      # yps2 (u): after recip1 read of den1
            if 1 <= gi <= 4:
                tgt = min(16, 4 * gi)
                while av2 < tgt:
                    emit_av_pair(2, av2)
                    av2 += 1
            if gi == 2:
                alloc_d(2)      # den2 (u): after the out1 add read of pps1
            if 3 <= gi <= 5:
                tgt = min(16, 6 * (gi - 2))
                while den2 < tgt:
                    emit_den_pair(2, den2)
                    den2 += 1
            if gi == 5:
                emit_epi_a(2)   # recip2 + ymul2 free den2/yps2 mid-block
            if gi == 6:
                emit_epi_b(2)   # pps2 (u): after ymul2 read of yps2
                alloc_y(3)      # yps3 (u): after recip2 read of den2
            if gi >= 7:
                ready = min(16, (3 * gi + 1) // 2 + 1)
                while av3 < ready:
                    emit_av_pair(3, av3)
                    av3 += 1
                if gi == 8:
                    alloc_d(3)  # den3 (u): after the out2 add read of pps2
                if gi >= 8:
                    while den3 < ready:
                        emit_den_pair(3, den3)
                        den3 += 1
        while av3 < 16:
            emit_av_pair(3, av3)
            av3 += 1
        while den3 < 16:
            emit_den_pair(3, den3)
            den3 += 1
        # tail epilogue in column halves to shorten the serial drain
        emit_epi_a(3, slice(0, 256), "a")
        emit_epi_b(3, slice(0, 256), "a")
        emit_epi_a(3, slice(256, 512), "b")
        emit_epi_b(3, slice(256, 512), "b")

    _split_excess_waits(nc)
    return nc


def _get_nc():
    if "nc" not in _CACHE:
        _CACHE["nc"] = _build_bass()
    return _CACHE["nc"]


def prepare_in_maps(x, gn_w, gn_b, wq, bq, wk, bk, wv, bv, wp, bp):
    import ml_dtypes

    bf = ml_dtypes.bfloat16
    f8 = ml_dtypes.float8_e4m3
    f32 = np.float32

    x = np.asarray(x, f32)
    xf = x.reshape(B, C, HW)

    def col(v):
        return np.ascontiguousarray(np.asarray(v, f32).reshape(C, 1))

    wq_t = np.ascontiguousarray(np.asarray(wq, f32).T).astype(bf)
    wk_t = np.ascontiguousarray(np.asarray(wk, f32).T).astype(bf)
    wv_t = np.ascontiguousarray(np.asarray(wv, f32).T).astype(bf)
    wp_t = np.ascontiguousarray(np.asarray(wp, f32).T).astype(bf)

    gmat = np.zeros((C, GROUPS), f32)
    for c in range(C):
        gmat[c, c // GSIZE] = 1.0
    gbc = np.ascontiguousarray(gmat.T * np.asarray(gn_w, f32)[None, :])
    gmat = gmat * f32(1.0 / NPIX_G)

    shared = {
        "wq_t": wq_t,
        "wk_t": wk_t,
        "wv_t": wv_t,
        "wp_t": wp_t,
        "bq": col(bq),
        "bv": col(bv),
        "bp": col(bp),
        "gn_b": col(gn_b),
        "gmat": gmat,
        "gbc": gbc,
        "ones2": np.ones((C, 2), f8),
    }

    in_maps = []
    for core in range(NCORES):
        b, qh = divmod(core, 2)
        if qh == 0:
            xp = np.ascontiguousarray(xf[b])
        else:
            xp = np.ascontiguousarray(
                np.concatenate([xf[b][:, HALF:], xf[b][:, :HALF]], axis=1)
            )
        in_maps.append(
            {
                "x": np.ascontiguousarray(xp[:, :HALF]),
                "x_bf": xp.astype(bf),
                **shared,
            }
        )
    return in_maps


def kernel(x, gn_w, gn_b, wq, bq, wk, bk, wv, bv, wp, bp):
    from concourse.bass_utils import run_bass_kernel_spmd

    f32 = np.float32
    in_maps = prepare_in_maps(x, gn_w, gn_b, wq, bq, wk, bk, wv, bv, wp, bp)
    nc = _get_nc()
    res = run_bass_kernel_spmd(nc, in_maps, core_ids=list(range(NCORES)))

    out = np.empty((B, C, HW), f32)
    for core in range(NCORES):
        b, qh = divmod(core, 2)
        out[b][:, HALF * qh : HALF * (qh + 1)] = res.results[core]["out"]
    return out.reshape(B, C, H, W)


# revision 28
# speedup vs baseline: 1.0456x; 1.0456x over previous
"""Self-contained Trainium2 Bass kernel for the BasicAttentionBlock problem.

Full inputs in, full outputs out. 8 NeuronCores, data-parallel over
(batch element x query-half): each core computes GroupNorm + q/k/v 1x1
convs + full-key attention for its 2048 query pixels + output projection
+ residual, entirely on-chip.

Design notes (v2):
- GroupNorm is folded into the conv weights on-chip (w' = w * a per input
  channel, conv biases recomputed from the GN shift b), so the convs
  consume raw x and the stats -> first-matmul chain is short. The k-conv
  bias cancels in softmax and is dropped.
- exp(S^T) on ACT (the bottleneck: 65536 columns/core) writes fp8 pT.
- AV and the softmax denominator are fp8 DoubleRow matmuls over key-block
  pairs (0.5 cycles/row): the denominator costs 16 matmuls/block on PE
  instead of a 31-add Pool tree, and AV halves.
- 1/den is broadcast across partitions by gpsimd; y is scaled by 1/den
  before the projection conv so the epilogue chain is short.
- PSUM: tag 'st' = 2 x [128,3,512] S^T groups (12KB), tag 'u' = 2 x 2KB
  rotating everything else (conv chunks, AV accumulators, denominators,
  projections) in a hand-ordered schedule that keeps the rotation free
  of slot deadlocks.
"""

import numpy as np

B = 4
C = 128
H = 64
W = 64
HW = H * W          # 4096
HALF = HW // 2      # 2048 query pixels per core
NCORES = 8
GROUPS = 8
GSIZE = C // GROUPS  # 16
EPS = 1e-5
SCL = 1.0 / np.sqrt(C)   # attention logit scale
NPIX_G = GSIZE * HW      # elements per group-norm group = 65536

_CACHE = {}


def _split_excess_waits(nc, limit=1):
    """Rewrite instructions so none carries more than `limit` sync-waits.

    The walrus build in this container rejects instructions with more than
    one sync-wait command ("Too many sync wait commands"), while Tile's
    semaphore assignment freely attaches several. Excess waits are hoisted
    onto standalone InstEventSemaphore instructions placed immediately
    before the owning instruction on the same engine queue — semantically
    identical (program order on one engine), just more instructions.
    """
    import concourse.mybir as mybir

    ctr = 0
    for f in nc.m.functions:
        for bb in f.blocks:
            new = []
            changed = False
            for inst in bb.instructions:
                si = getattr(inst, "sync_info", None)
                ow = list(si.on_wait) if si is not None else []
                if len(ow) > limit:
                    # keep register-valued waits on the original instruction
                    imm = [w for w in ow if w.wait_reg is None]
                    reg = [w for w in ow if w.wait_reg is not None]
                    keep_n = max(0, limit - len(reg))
                    hoist = imm[: len(imm) - keep_n] if keep_n < len(imm) else []
                    kept = reg + imm[len(imm) - keep_n :] if keep_n else reg
                    assert len(kept) <= max(limit, len(reg))
                    for w in hoist:
                        ev = mybir.InstEventSemaphore(
                            name=f"waitsplit_{ctr}", ins=[], outs=[]
                        )
                        ctr += 1
                        ev.engine = inst.engine
                        ev.sync_info = mybir.SyncInfo(on_wait=[w], on_update=[])
                        nc.register_instruction(ev, overwrite=True)
                        new.append(ev)
                    si.on_wait = kept
                    inst.sync_info = si
                    changed = True
                new.append(inst)
            if changed:
                bb.instructions = new


def _build_bass():
    import concourse.bass as bass
    import concourse.mybir as mybir

    fp32 = mybir.dt.float32
    bf16 = mybir.dt.bfloat16
    fp8 = mybir.dt.float8e4
    AF = mybir.ActivationFunctionType
    ALU = mybir.AluOpType
    AX = mybir.AxisListType
    DR = mybir.MatmulPerfMode.DoubleRow
    from concourse.tile import TileContext as TC

    nc = bass.Bass(trn_type="TRN2")

    # ---- I/O -----------------------------------------------------------
    x_d = nc.dram_tensor("x", [C, HALF], fp32, kind="ExternalInput")
    xbf_d = nc.dram_tensor("x_bf", [C, HW], bf16, kind="ExternalInput")
    wq_d = nc.dram_tensor("wq_t", [C, C], bf16, kind="ExternalInput")
    wk_d = nc.dram_tensor("wk_t", [C, C], bf16, kind="ExternalInput")
    wv_d = nc.dram_tensor("wv_t", [C, C], bf16, kind="ExternalInput")
    wp_d = nc.dram_tensor("wp_t", [C, C], bf16, kind="ExternalInput")
    bq_d = nc.dram_tensor("bq_row", [1, C], fp32, kind="ExternalInput")
    bv_d = nc.dram_tensor("bv", [C, 1], fp32, kind="ExternalInput")
    bp_d = nc.dram_tensor("bp", [C, 1], fp32, kind="ExternalInput")
    gnb_d = nc.dram_tensor("gn_b", [C, 1], fp32, kind="ExternalInput")
    gmat_d = nc.dram_tensor("gmat", [C, GROUPS], fp32, kind="ExternalInput")
    gbc_d = nc.dram_tensor("gbc", [GROUPS, C], fp32, kind="ExternalInput")
    ones2_d = nc.dram_tensor("ones2", [C, 2], fp8, kind="ExternalInput")
    out_d = nc.dram_tensor("out", [C, HALF], fp32, kind="ExternalOutput")

    with TC(nc) as tc, tc.tile_pool(name="main", bufs=1) as pool, tc.tile_pool(
        name="psum", bufs=1, space="PSUM"
    ) as psum:
        # gpsimd 'proxy' library: tensor_tensor + partition_broadcast
        from concourse import library_config

        nc.gpsimd.load_library(library_config.proxy)

        # ---- ACT table prewarm (hide the exp table load) ---------------
        dum = pool.tile([1, 2], fp32, name="dum")
        nc.vector.memset(dum[:], 0.0)

        # ---- SBUF tiles -------------------------------------------------
        x_bf = pool.tile([C, HW], bf16, name="x_bf")
        x_sb = pool.tile([C, HALF], fp32, name="x_sb")
        wq_sb = pool.tile([C, C], bf16, name="wq_sb")
        wk_sb = pool.tile([C, C], bf16, name="wk_sb")
        wv_sb = pool.tile([C, C], bf16, name="wv_sb")
        wp_sb = pool.tile([C, C], bf16, name="wp_sb")
        wqs_sb = pool.tile([C, C], bf16, name="wqs_sb")
        wks_sb = pool.tile([C, C], bf16, name="wks_sb")
        wvs_sb = pool.tile([C, C], bf16, name="wvs_sb")
        bq_sb = pool.tile([1, C], fp32, name="bq_sb")
        bv_sb = pool.tile([C, 1], fp32, name="bv_sb")
        bp_sb = pool.tile([C, 1], fp32, name="bp_sb")
        gnb_sb = pool.tile([C, 1], fp32, name="gnb_sb")
        gmat_sb = pool.tile([C, GROUPS], fp32, name="gmat_sb")
        gbc_sb = pool.tile([GROUPS, C], fp32, name="gbc_sb")
        ones2_sb = pool.tile([C, 2], fp8, name="ones2_sb")

        # ---- DMAs -------------------------------------------------------
        # SP queue: x_bf chunks 0-3 (stats-critical), then weights/consts
        # in order of first use; ACT queue: x_bf 4-5; Pool queue: x_bf
        # 6-7, then the late consts + residual x.
        for c4 in range(2):
            sl = slice(1024 * c4, 1024 * (c4 + 1))
            nc.sync.dma_start(x_bf[:, sl], xbf_d[:, sl])
        nc.scalar.dma_start(x_bf[:, 2048:3072], xbf_d[:, 2048:3072])
        nc.gpsimd.dma_start(x_bf[:, 3072:4096], xbf_d[:, 3072:4096])
        nc.sync.dma_start(wq_sb[:], wq_d[:])
        nc.sync.dma_start(wk_sb[:], wk_d[:])
        nc.sync.dma_start(gmat_sb[:], gmat_d[:])
        nc.sync.dma_start(gbc_sb[:], gbc_d[:])
        nc.sync.dma_start(gnb_sb[:], gnb_d[:])
        nc.sync.dma_start(bq_sb[:], bq_d[:])
        nc.sync.dma_start(wv_sb[:], wv_d[:])
        nc.sync.dma_start(wp_sb[:], wp_d[:])
        # prewarm exp/ln table while the stats DMAs stream
        nc.scalar.activation(dum[:], dum[:], AF.Exp)
        nc.sync.dma_start(bv_sb[:], bv_d[:])
        nc.sync.dma_start(bp_sb[:], bp_d[:])
        nc.sync.dma_start(ones2_sb[:], ones2_d[:])
        for c4 in range(4):
            sl = slice(512 * c4, 512 * (c4 + 1))
            nc.sync.dma_start(x_sb[:, sl], x_d[:, sl])

        # ---- GroupNorm stats (chunked, overlaps the x DMA) -------------
        # per-channel sums on Pool (idle, no access-latency penalty);
        # per-channel sum of squares: first half on ACT (Square + accum,
        # shares the exp table set), second half on DVE (ttr).
        s_parts = pool.tile([C, 8], fp32, name="s_parts")
        ss_parts = pool.tile([C, 2], fp32, name="ss_parts")
        sq_scr = pool.tile([C, 2048], bf16, name="sq_scr")
        tt_scr = pool.tile([C, 2048], bf16, name="tt_scr")
        s_scr = pool.tile([C, 512], bf16, name="s_scr")
        stats = pool.tile([C, 2], fp32, name="stats")
        def pool_sum(ci):
            sl = slice(512 * ci, 512 * (ci + 1))
            nc.gpsimd.tensor_scalar(
                s_scr[:],
                x_bf[:, sl],
                0.0,
                None,
                ALU.add,
                ALU.add,
                accum_out=s_parts[:, ci : ci + 1],
            )

        for ci in (6, 7, 4, 5, 0, 1):  # in order of chunk arrival
            pool_sum(ci)
        nc.scalar.activation(
            sq_scr[:], x_bf[:, 0:2048], AF.Square, accum_out=ss_parts[:, 0:1]
        )
        nc.vector.tensor_tensor_reduce(
            tt_scr[:],
            x_bf[:, 2048:4096],
            x_bf[:, 2048:4096],
            1.0,
            0.0,
            ALU.mult,
            ALU.add,
            accum_out=ss_parts[:, 1:2],
        )
        # last two sums on DVE (Pool is the stats throughput pole)
        for ci in (2, 3):
            sl = slice(512 * ci, 512 * (ci + 1))
            nc.vector.tensor_scalar(
                tt_scr[:, 0:512],
                x_bf[:, sl],
                0.0,
                None,
                ALU.add,
                ALU.add,
                accum_out=s_parts[:, ci : ci + 1],
            )
        nc.vector.tensor_reduce(stats[:, 0:1], s_parts[:], axis=AX.X, op=ALU.add)
        nc.vector.tensor_reduce(stats[:, 1:2], ss_parts[:], axis=AX.X, op=ALU.add)

        eps_sb = pool.tile([GROUPS, 1], fp32, name="eps_sb")
        nc.vector.memset(eps_sb[:], EPS)

        gsum_ps = psum.tile([GROUPS, 2], fp32, name="gsum_ps", tag="u", bufs=2)
        nc.tensor.matmul(gsum_ps[:], gmat_sb[:], stats[:], start=True, stop=True)
        me2 = pool.tile([GROUPS, 2], fp32, name="me2")
        nc.vector.tensor_copy(me2[:], gsum_ps[:])

        msq = pool.tile([GROUPS, 1], fp32, name="msq")
        nc.vector.tensor_tensor(msq[:], me2[:, 0:1], me2[:, 0:1], ALU.mult)
        tve = pool.tile([GROUPS, 1], fp32, name="tve")
        nc.vector.tensor_tensor(tve[:], me2[:, 1:2], msq[:], ALU.subtract)

        # rsqrt(var+eps) = exp(-0.5*ln(var+eps)); eps rides the Ln bias.
        lnt = pool.tile([GROUPS, 1], fp32, name="lnt")
        nc.scalar.activation(lnt[:], tve[:], AF.Ln, bias=eps_sb[:])
        r1 = pool.tile([GROUPS, 1], fp32, name="r1")
        nc.scalar.activation(r1[:], lnt[:], AF.Exp, scale=-0.5)
        mr = pool.tile([GROUPS, 1], fp32, name="mr")
        nc.vector.tensor_tensor(mr[:], me2[:, 0:1], r1[:], ALU.mult)

        # a = gn_w * rsqrt (per channel), b = gn_b - mean * a
        a_ps = psum.tile([C, 1], fp32, name="a_ps", tag="u", bufs=2)
        nc.tensor.matmul(a_ps[:], gbc_sb[:], r1[:], start=True, stop=True)
        bm_ps = psum.tile([C, 1], fp32, name="bm_ps", tag="u", bufs=2)
        nc.tensor.matmul(bm_ps[:], gbc_sb[:], mr[:], start=True, stop=True)
        a_sb = pool.tile([C, 1], fp32, name="a_sb")
        nc.vector.tensor_copy(a_sb[:], a_ps[:])
        b_sb = pool.tile([C, 1], fp32, name="b_sb")
        nc.vector.tensor_tensor(b_sb[:], gnb_sb[:], bm_ps[:], ALU.subtract)

        # fold the GN scale into the conv weights: w'[c,o] = w_t[c,o]*a[c]
        nc.vector.tensor_scalar(wqs_sb[:], wq_sb[:], a_sb[:], None, ALU.mult)
        nc.vector.tensor_scalar(wks_sb[:], wk_sb[:], a_sb[:], None, ALU.mult)
        nc.vector.tensor_scalar(wvs_sb[:], wv_sb[:], a_sb[:], None, ALU.mult)
        b_bf = pool.tile([C, 1], bf16, name="b_bf")
        nc.vector.tensor_copy(b_bf[:], b_sb[:])
        # q-conv bias as a ROW (one matmul: out[0,o] = sum_c b[c] wq[c,o]),
        # later accumulated into each q-conv psum as a rank-1 matmul so the
        # q evacs are plain copies with no scalar-bias ordering hazard.
        bhqr_ps = psum.tile([1, C], fp32, name="bhqr_ps", tag="u", bufs=2)
        nc.tensor.matmul(bhqr_ps[:], b_bf[:], wq_sb[:], start=True, stop=True)
        bhq_row = pool.tile([1, C], bf16, name="bhq_row")
        nc.vector.tensor_tensor(bhq_row[:], bhqr_ps[:], bq_sb[:], ALU.add)
        ones_row = pool.tile([1, 512], bf16, name="ones_row")
        nc.vector.memset(ones_row[:], 1.0)

        def emit_biases():
            """v/p conv biases from the GN shift b (emitted after the first
            q/k conv matmuls so PE serves those first):
              bhv = wv.b + bv ; bp2 = wp.bhv + bp
            (the k-conv bias is constant per query in the logits -> cancels)
            """
            bhv_ps = psum.tile([C, 1], fp32, name="bhv_ps", tag="u", bufs=2)
            nc.tensor.matmul(bhv_ps[:], wv_sb[:], b_bf[:], start=True, stop=True)
            bhv_sb = pool.tile([C, 1], fp32, name="bhv_sb")
            nc.vector.tensor_tensor(bhv_sb[:], bhv_ps[:], bv_sb[:], ALU.add)
            bhv_bf = pool.tile([C, 1], bf16, name="bhv_bf")
            nc.vector.tensor_copy(bhv_bf[:], bhv_sb[:])
            pb_ps = psum.tile([C, 1], fp32, name="pb_ps", tag="u", bufs=2)
            nc.tensor.matmul(pb_ps[:], wp_sb[:], bhv_bf[:], start=True, stop=True)
            bp2_sb = pool.tile([C, 1], fp32, name="bp2_sb")
            nc.vector.tensor_tensor(bp2_sb[:], pb_ps[:], bp_sb[:], ALU.add)
            # xb = x + bp2 (residual + folded projection bias), on Pool
            nc.gpsimd.tensor_scalar(xb[:], x_sb[:], bp2_sb[:], None, ALU.add)

        xb = pool.tile([C, HALF], fp32, name="xb")

        # ---- conv emitters ---------------------------------------------
        k_bf = pool.tile([C, HW], bf16, name="k_bf")
        q_bf = pool.tile([C, HALF], bf16, name="q_bf")
        vT_f8 = pool.tile([C, 32, C], fp8, name="vT_f8")

        def emit_k_chunk(c8):
            sl = slice(512 * c8, 512 * (c8 + 1))
            kps = psum.tile([C, 512], fp32, name=f"kps{c8}", tag="u", bufs=2)
            nc.tensor.matmul(kps[:], wks_sb[:], x_bf[:, sl], start=True, stop=True)
            nc.vector.tensor_copy(k_bf[:, sl], kps[:])

        def emit_q_chunk(c4):
            sl = slice(512 * c4, 512 * (c4 + 1))
            qps = psum.tile([C, 512], fp32, name=f"qps{c4}", tag="u", bufs=2)
            nc.tensor.matmul(qps[:], wqs_sb[:], x_bf[:, sl], start=True, stop=False)
            nc.tensor.matmul(qps[:], bhq_row[:], ones_row[:], start=False, stop=True)
            if c4 == 0:
                # ACT is idle pre-body: overlap the q evac with the k evac
                nc.scalar.activation(q_bf[:, sl], qps[:], AF.Copy)
            else:
                nc.vector.tensor_copy(q_bf[:, sl], qps[:])

        def emit_v_chunk(g8):
            vps = psum.tile([C, 512], fp32, name=f"vps{g8}", tag="u", bufs=2)
            for m in range(4):
                jb = 4 * g8 + m
                nc.tensor.matmul(
                    vps[:, 128 * m : 128 * (m + 1)],
                    x_bf[:, 128 * jb : 128 * (jb + 1)],
                    wvs_sb[:],
                    start=True,
                    stop=True,
                )
            nc.vector.tensor_copy(
                vT_f8[:, 4 * g8 : 4 * (g8 + 1), :],
                vps[:].rearrange("p (m c) -> p m c", m=4),
            )

        emit_q_chunk(0)
        emit_k_chunk(0)
        emit_biases()

        # ---- attention --------------------------------------------------
        jgroups = [(3 * g, 3) for g in range(10)] + [(30, 2)]
        n_ib = HALF // 512  # 4 query blocks of 512
        pT_tiles = [None] * n_ib
        yps_tiles = [None] * n_ib
        den_tiles = [None] * n_ib
        out_sb = pool.tile([C, HALF], fp32, name="out_sb")
        ones_ap = ones2_sb[:].unsqueeze(2)  # [128, 2, 1]

        conv_state = {"k": 1, "q": 1, "v": 0}
        # conv chunk emission schedule for block 0 (group -> jobs).
        blk0_jobs = {
            0: ["k", "v"], 1: ["k", "v"], 2: ["k", "v"], 3: ["k", "q", "v"],
            4: ["k", "v"], 5: ["k", "q", "v"], 6: ["k", "v"], 7: ["k", "q", "v"],
        }

        def run_conv_job(j):
            if j == "k" and conv_state["k"] < 8:
                emit_k_chunk(conv_state["k"])
                conv_state["k"] += 1
            elif j == "q" and conv_state["q"] < 4:
                emit_q_chunk(conv_state["q"])
                conv_state["q"] += 1
            elif j == "v" and conv_state["v"] < 8:
                emit_v_chunk(conv_state["v"])
                conv_state["v"] += 1

        def alloc_y(ib):
            yps_tiles[ib] = psum.tile([C, 512], fp32, name=f"yps{ib}", tag="u", bufs=2)

        def alloc_d(ib):
            den_tiles[ib] = psum.tile([1, 512], fp32, name=f"den{ib}", tag="u", bufs=2)

        def emit_av_pair(ib, p):
            nc.tensor.matmul(
                yps_tiles[ib][:],
                vT_f8[:, 2 * p : 2 * p + 2, :],
                pT_tiles[ib][:, 2 * p : 2 * p + 2, :],
                start=(p == 0),
                stop=(p == 15),
                perf_mode=DR,
            )

        def emit_den_pair(ib, p):
            nc.tensor.matmul(
                den_tiles[ib][:],
                ones_ap,
                pT_tiles[ib][:, 2 * p : 2 * p + 2, :],
                start=(p == 0),
                stop=(p == 15),
                perf_mode=DR,
            )

        rbc_tiles = [None] * n_ib
        ybf_tiles = [None] * n_ib

        def emit_epi_a(ib, hs=slice(0, 512), tag=""):
            """den -> 1/den -> partition-broadcast -> y*(1/den): emitted at
            the end of the block that ran AV(ib)/den(ib)."""
            n = hs.stop - hs.start
            rden = pool.tile([1, n], fp32, name=f"rden{ib}{tag}", tag="rden", bufs=2)
            nc.vector.reciprocal(rden[:], den_tiles[ib][:, hs])
            rbc = pool.tile([C, n], fp32, name=f"rbc{ib}{tag}", tag="rbc", bufs=2)
            nc.gpsimd.partition_broadcast(rbc[:], rden[:])
            y_bf = pool.tile([C, n], bf16, name=f"ybf{ib}{tag}", tag="ybf", bufs=2)
            nc.vector.tensor_tensor(y_bf[:], yps_tiles[ib][:, hs], rbc[:], ALU.mult)
            ybf_tiles[ib] = y_bf

        def emit_epi_b(ib, hs=slice(0, 512), tag=""):
            """proj -> + (x + bp2) -> store."""
            n = hs.stop - hs.start
            sl = slice(512 * ib + hs.start, 512 * ib + hs.stop)
            pps = psum.tile([C, n], fp32, name=f"pps{ib}{tag}", tag="u", bufs=2)
            nc.tensor.matmul(pps[:], wp_sb[:], ybf_tiles[ib][:], start=True, stop=True)
            nc.vector.tensor_tensor(out_sb[:, sl], pps[:], xb[:, sl], ALU.add)
            nc.sync.dma_start(out_d[:, sl], out_sb[:, sl])

        def emit_st_group(ib, j0, glen):
            st = psum.tile([C, glen, 512], fp32, name=f"st{ib}_{j0}", tag="st", bufs=2)
            qs = q_bf[:, 512 * ib : 512 * (ib + 1)]
            for u2 in range(glen):
                jb = j0 + u2
                nc.tensor.matmul(
                    st[:, u2, :],
                    k_bf[:, 128 * jb : 128 * (jb + 1)],
                    qs,
                    start=True,
                    stop=True,
                )
            nc.scalar.activation(
                pT_tiles[ib][:, j0 : j0 + glen, :], st[:], AF.Exp, scale=float(SCL)
            )

        # -- block 0: S^T/exp + remaining conv chunks ---------------------
        pT_tiles[0] = pool.tile([C, 32, 512], fp8, name="pT0", tag="pT", bufs=3)
        for gi, (j0, glen) in enumerate(jgroups):
            emit_st_group(0, j0, glen)
            for j in blk0_jobs.get(gi, []):
                run_conv_job(j)

        # -- block 1: + AV0/den0 spread over the block --------------------
        pT_tiles[1] = pool.tile([C, 32, 512], fp8, name="pT1", tag="pT", bufs=3)
        alloc_y(0)
        alloc_d(0)
        av_done = den_done = 0
        for gi, (j0, glen) in enumerate(jgroups):
            emit_st_group(1, j0, glen)
            if gi >= 1:
                tgt = min(16, 2 * gi)
                while av_done < tgt:
                    emit_av_pair(0, av_done)
                    av_done += 1
                while den_done < tgt:
                    emit_den_pair(0, den_done)
                    den_done += 1
        while av_done < 16:
            emit_av_pair(0, av_done)
            av_done += 1
        while den_done < 16:
            emit_den_pair(0, den_done)
            den_done += 1
        emit_epi_a(0)

        # -- block 2: + epilogue-B(0), AV1/den1 spread --------------------
        pT_tiles[2] = pool.tile([C, 32, 512], fp8, name="pT2", tag="pT", bufs=3)
        av_done = den_done = 0
        for gi, (j0, glen) in enumerate(jgroups):
            emit_st_group(2, j0, glen)
            if gi == 0:
                emit_epi_b(0)   # pps0 (u): after ymul0 read of yps0
                alloc_y(1)      # yps1 (u): after recip0 read of den0
            if gi >= 1:
                tgt = min(16, 2 * gi)
                while av_done < tgt:
                    emit_av_pair(1, av_done)
                    av_done += 1
            if gi == 2:
                alloc_d(1)      # den1 (u): after the out0 add read of pps0
            if gi >= 3:
                tgt = min(16, 3 * (gi - 2))
                while den_done < tgt:
                    emit_den_pair(1, den_done)
                    den_done += 1
        while av_done < 16:
            emit_av_pair(1, av_done)
            av_done += 1
        while den_done < 16:
            emit_den_pair(1, den_done)
            den_done += 1
        emit_epi_a(1)

        # -- block 3: epi-B(1), AV2/den2 bursts, self-trailing AV3/den3 --
        pT_tiles[3] = pool.tile([C, 32, 512], fp8, name="pT3", tag="pT", bufs=3)
        av2 = den2 = 0
        av3 = den3 = 0
        for gi, (j0, glen) in enumerate(jgroups):
            emit_st_group(3, j0, glen)
            if gi == 0:
                emit_epi_b(1)   # pps1 (u): after ymul1 read of yps1
                alloc_y(2)      # yps2 (u): after recip1 read of den1
            if 1 <= gi <= 4:
                tgt = min(16, 4 * gi)
                while av2 < tgt:
                    emit_av_pair(2, av2)
                    av2 += 1
            if gi == 2:
                alloc_d(2)      # den2 (u): after the out1 add read of pps1
            if 3 <= gi <= 5:
                tgt = min(16, 6 * (gi - 2))
                while den2 < tgt:
                    emit_den_pair(2, den2)
                    den2 += 1
            if gi == 5:
                emit_epi_a(2)   # recip2 + ymul2 free den2/yps2 mid-block
            if gi == 6:
                emit_epi_b(2)   # pps2 (u): after ymul2 read of yps2
                alloc_y(3)      # yps3 (u): after recip2 read of den2
            if gi >= 7:
                ready = min(16, (3 * gi + 1) // 2 + 1)
                while av3 < ready:
                    emit_av_pair(3, av3)
                    av3 += 1
                if gi == 8:
                    alloc_d(3)  # den3 (u): after the out2 add read of pps2
                if gi >= 8:
                    while den3 < ready:
                        emit_den_pair(3, den3)
                        den3 += 1
        while av3 < 16:
            emit_av_pair(3, av3)
            av3 += 1
        while den3 < 16:
            emit_den_pair(3, den3)
            den3 += 1
        # tail epilogue in column halves to shorten the serial drain
        emit_epi_a(3, slice(0, 256), "a")
        emit_epi_b(3, slice(0, 256), "a")
        emit_epi_a(3, slice(256, 512), "b")
        emit_epi_b(3, slice(256, 512), "b")

    _split_excess_waits(nc)
    return nc


def _get_nc():
    if "nc" not in _CACHE:
        _CACHE["nc"] = _build_bass()
    return _CACHE["nc"]


def prepare_in_maps(x, gn_w, gn_b, wq, bq, wk, bk, wv, bv, wp, bp):
    import ml_dtypes

    bf = ml_dtypes.bfloat16
    f8 = ml_dtypes.float8_e4m3
    f32 = np.float32

    x = np.asarray(x, f32)
    xf = x.reshape(B, C, HW)

    def col(v):
        return np.ascontiguousarray(np.asarray(v, f32).reshape(C, 1))

    wq_t = np.ascontiguousarray(np.asarray(wq, f32).T).astype(bf)
    wk_t = np.ascontiguousarray(np.asarray(wk, f32).T).astype(bf)
    wv_t = np.ascontiguousarray(np.asarray(wv, f32).T).astype(bf)
    wp_t = np.ascontiguousarray(np.asarray(wp, f32).T).astype(bf)

    gmat = np.zeros((C, GROUPS), f32)
    for c in range(C):
        gmat[c, c // GSIZE] = 1.0
    gbc = np.ascontiguousarray(gmat.T * np.asarray(gn_w, f32)[None, :])
    gmat = gmat * f32(1.0 / NPIX_G)

    shared = {
        "wq_t": wq_t,
        "wk_t": wk_t,
        "wv_t": wv_t,
        "wp_t": wp_t,
        "bq_row": np.ascontiguousarray(np.asarray(bq, f32).reshape(1, C)),
        "bv": col(bv),
        "bp": col(bp),
        "gn_b": col(gn_b),
        "gmat": gmat,
        "gbc": gbc,
        "ones2": np.ones((C, 2), f8),
    }

    in_maps = []
    for core in range(NCORES):
        b, qh = divmod(core, 2)
        if qh == 0:
            xp = np.ascontiguousarray(xf[b])
        else:
            xp = np.ascontiguousarray(
                np.concatenate([xf[b][:, HALF:], xf[b][:, :HALF]], axis=1)
            )
        in_maps.append(
            {
                "x": np.ascontiguousarray(xp[:, :HALF]),
                "x_bf": xp.astype(bf),
                **shared,
            }
        )
    return in_maps


def kernel(x, gn_w, gn_b, wq, bq, wk, bk, wv, bv, wp, bp):
    from concourse.bass_utils import run_bass_kernel_spmd

    f32 = np.float32
    in_maps = prepare_in_maps(x, gn_w, gn_b, wq, bq, wk, bk, wv, bv, wp, bp)
    nc = _get_nc()
    res = run_bass_kernel_spmd(nc, in_maps, core_ids=list(range(NCORES)))

    out = np.empty((B, C, HW), f32)
    for core in range(NCORES):
        b, qh = divmod(core, 2)
        out[b][:, HALF * qh : HALF * (qh + 1)] = res.results[core]["out"]
    return out.reshape(B, C, H, W)
